# revision 22
# baseline (speedup 1.0000x reference)
"""Dense bilateral energy loss (DenseEnergyLoss) on 8 Trainium2 cores.

Math (per image n, after 2x downsample => oh=ow=64, P=4096):
  feat[p] = (x/40, y/40, r/15, g/15, b/15)          # 5 dims
  A[p,q]  = exp(-(||feat_p - feat_q||^2)/2)          # dense [P,P]
  AS[k,q] = sum_p seg_m[k,p] * A[p,q]                # A symmetric
  loss    = -0.05 * sum_{k,q} seg_m[k,q]*gate[q]*AS[k,q] / (N*P)

Device work per core (half an image: 2048 of the 4096 q columns):
  MM1 (PE):  dot[p,q] = -0.5*d2[p,q] via bf16 hi/lo-split contraction,
             zero-padded 21->128 rows (keeps PE HAM at 2.4GHz; K=32 without
             padding measured: HAM never unthrottles, PE stuck at 1.2GHz;
             4x row-tiled K=32 also measured: HAM stays cold and the row
             tiles mostly serialize -- net loss).
  EXP: split across two engines per [128,1024] dot tile:
    ACT: A = exp(dot) -> bf16  (exact spline)
    DVE: Schraudolph bit-trick: u16 = sat_round(dot*128*log2e + 128*(127-s));
         those 16 bits ARE the bf16 A up to a mean-zero sawtooth (<2e-3 on
         the final loss after calibrating s; verified in numpy + HW probe:
         the DVE fp32->uint16 converter rounds-to-nearest and saturates).
  MM2 (PE):  AS^T accumulation, col-tiled 4x (tile_position=(0,32j)).
  Per band: ACT copies the 4 col-group partials [117,512] PSUM->SBUF, DMA'd
  out; HOST sums the 4 partials (removes the DVE add-chain + fat tail).
Host (numpy): resizes, gate, seg_m, features, hi/lo split, partial sums,
final masked reduction.
"""

import sys

sys.path.insert(0, "/opt/trn_rl_repo")

import numpy as np
import ml_dtypes

# ---------------- problem constants (hardcoded per contract) ---------------
N, K, H, W = 4, 21, 128, 128
OH, OW = 64, 64
P = OH * OW  # 4096
WEIGHT = 0.1
SIGMA_RGB = 15.0
SIGMA_XY = 80.0
SCALE = 0.5
IGNORE_LABEL = 255
N_CORES = 8
QCOLS = P // 2  # q columns per core (2 cores per image)
QB = 512  # q tile width (one PSUM bank)
NQ = QCOLS // QB  # 4 q-bands per core
NPB = P // 128  # 32 p-blocks
CROWS = 128  # MM1 contraction rows (21 real + zero padding)
ROW_TILED_MM1 = False  # 4 concurrent K=32 row tiles (needs 4x replicated rows)

# Schraudolph constants (calibrated in numpy on the real inputs)
SCH_SIGMA = 0.0565
SCH_C1 = 128.0 / float(np.log(2.0))
SCH_C2 = 128.0 * (127.0 - SCH_SIGMA)

BF16 = ml_dtypes.bfloat16

_PROGRAM = None  # built once per process


def _dve_tile(qb, pg, half):
    """Which dot tiles the DVE (Schraudolph) handles vs ACT (exact exp).
    DVE gets half==1 except two tiles given back to ACT for balance."""
    if half != 0:
        return False
    if pg == 7 and qb in (1, 2):
        return False
    return True


def _hilo(x):
    """Split fp32 array into bf16 hi + bf16 lo with x ~= hi + lo."""
    x = np.asarray(x, np.float32)
    hi = x.astype(BF16)
    lo = (x - hi.astype(np.float32)).astype(BF16)
    return hi, lo


def _patch_tile_drain():
    """This container's walrus allows only one sync wait per CTRL (Drain/Nop)
    instruction; Tile's exit drain attaches one wait per DMA-HW queue sem.
    Split the extra waits onto dedicated nops."""
    from concourse import mybir
    from concourse.tile import TileContext
    from concourse.vector_clock import ScopedClock

    if getattr(TileContext, "_drain_split_patched", False):
        return

    def _drain_and_barrier(self, tick_clock, wait_clock):
        nc = self.nc
        drain_inst = nc.sync.drain()
        wait_clock.add_sem_waits(
            drain_inst.ins, ScopedClock({None: tick_clock.global_clock})
        )
        si = drain_inst.ins.sync_info
        waits = list(si.on_wait) if si is not None else []
        if len(waits) > 1:
            del si.on_wait[1:]
            for w in waits[1:]:
                n = nc.sync.nop(nofuse=True, hint="drain_split")
                n.ins.sync_info = mybir.SyncInfo(on_wait=[w], on_update=[])
        nc.all_engine_barrier()
        popped = nc._tile_sem_poison_stack.pop()
        assert popped is self._sem_poison
        nc.clear_and_free_semaphores(list(self.sems.allocated().values()))
        nc.all_engine_barrier()

    TileContext._drain_and_barrier = _drain_and_barrier
    TileContext._drain_split_patched = True


def _split_multi_waits(nc):
    """This walrus build supports one sync-wait per instruction. Hoist extra
    waits onto dedicated same-engine nops placed right before the owner."""
    from concourse import mybir

    ctr = 0
    for fn in nc.m.functions:
        for blk in fn.blocks:
            insts = blk.instructions
            new = []
            changed = False
            for inst in insts:
                si = inst.sync_info
                if si is not None and si.on_wait is not None and len(si.on_wait) > 1:
                    waits = list(si.on_wait)
                    for w in waits[:-1]:
                        ctr += 1
                        new.append(
                            mybir.InstNoOp(
                                name=f"WSPLIT-{ctr}",
                                engine=inst.engine,
                                ins=[],
                                outs=[],
                                sync_info=mybir.SyncInfo(
                                    on_wait=[w], on_update=[]
                                ),
                                text_hint="wait_split",
                                bass_nofuse=True,
                            )
                        )
                    si.on_wait = [waits[-1]]
                    inst.sync_info = si
                    changed = True
                new.append(inst)
            if changed:
                blk.instructions = new


def _build_program():
    global _PROGRAM
    if _PROGRAM is not None:
        return _PROGRAM

    _patch_tile_drain()
    import concourse.bass as bass
    from concourse import mybir
    from concourse.tile import TileContext

    nc = bass.Bass("TRN2")
    f32 = mybir.dt.float32
    bf16 = mybir.dt.bfloat16
    u16 = mybir.dt.uint16

    # Full 128 contraction rows (21 real + 107 host-zeroed): 128-partition
    # DMAs engage all 16 SDMA engines (32/96-row transfers measured ~4x
    # slower), and the zero pad rows keep the PE HAM activity monitor at
    # full clock.
    flt = nc.dram_tensor("flt", [CROWS, P], bf16, kind="ExternalInput")
    frt = nc.dram_tensor("frt", [CROWS, QCOLS], bf16, kind="ExternalInput")
    # seg_m^T pre-arranged [128, NPB*21]: st[p, pb*21+k] = seg_m[k, pb*128+p]
    st = nc.dram_tensor("st", [128, NPB * 21], bf16, kind="ExternalInput")
    # per-band 4 col-group partials [117, QB], host sums rows 0:21,32:53,...
    out4 = nc.dram_tensor("out4", [NQ * 117, QB], f32, kind="ExternalOutput")

    with TileContext(nc) as tc:
        with (
            tc.tile_pool(name="const", bufs=1) as const,
            tc.tile_pool(name="apool", bufs=10) as apool,
            tc.tile_pool(name="osb", bufs=2) as osb,
            tc.tile_pool(name="dotps", bufs=3, space="PSUM") as dotps,
            tc.tile_pool(name="outps", bufs=2, space="PSUM") as outps,
        ):
            flt_s = const.tile([CROWS, P], bf16)
            frt_s = const.tile([CROWS, QCOLS], bf16)
            st_s = const.tile([128, NPB * 21], bf16)
            # Inputs on two HWDGE rings in parallel (sync + scalar),
            # column-chunked so the first band/p-blocks land earliest.
            nc.sync.dma_start(out=frt_s[:, 0:QB], in_=frt[:, 0:QB])
            nc.sync.dma_start(out=st_s, in_=st[:, :])
            nc.sync.dma_start(out=frt_s[:, QB:QCOLS], in_=frt[:, QB:QCOLS])
            nc.scalar.dma_start(out=flt_s[:, 0:512], in_=flt[:, 0:512])
            nc.scalar.dma_start(out=flt_s[:, 512:2048], in_=flt[:, 512:2048])
            nc.scalar.dma_start(out=flt_s[:, 2048:P], in_=flt[:, 2048:P])

            def emit_mm2(out_ps, pg, a_pair):
                for j in range(4):  # col-tiled MM2, 4 p-blocks at once
                    pb = pg * 4 + j
                    nc.tensor.matmul(
                        out_ps[32 * j : 32 * j + 21, :],
                        lhsT=st_s[:, pb * 21 : (pb + 1) * 21],
                        rhs=a_pair[j // 2][:, (j % 2) * QB : (j % 2 + 1) * QB],
                        tile_position=(0, 32 * j),
                        start=(pg == 0),
                        stop=(pg == NPB // 4 - 1),
                    )

            def emit_band_out(qb, out_ps):
                # Evacuate the 4 col-group partials via ACT (close to PSUM);
                # host sums them. DMA on the sync ring (idle mid-loop).
                # Last band: split by ROWS across ACT and DVE in parallel
                # (contiguous DMA destinations; a column split was measured
                # 2.5us DMA-completion due to strided rows, gating the drain).
                if qb == NQ - 1:
                    o_sb1 = osb.tile([96, QB], f32, name="o_sb1", tag="of1")
                    nc.scalar.copy(o_sb1, out_ps[0:96, :])
                    nc.sync.dma_start(
                        out=out4[qb * 117 : qb * 117 + 96, :], in_=o_sb1
                    )
                    o_sb2 = osb.tile([21, QB], f32, name="o_sb2", tag="of2")
                    nc.vector.tensor_copy(o_sb2, out_ps[96:117, :])
                    nc.sync.dma_start(
                        out=out4[qb * 117 + 96 : (qb + 1) * 117, :], in_=o_sb2
                    )
                else:
                    o_sb = osb.tile(
                        [117, QB], f32, name="o_sb", tag=f"o{qb % 2}"
                    )
                    # qb 1,2 evacuate via DVE to balance engine load (ACT
                    # carries 34 exps vs DVE 30).
                    if qb in (1, 2):
                        nc.vector.tensor_copy(o_sb, out_ps[0:117, :])
                    else:
                        nc.scalar.copy(o_sb, out_ps[0:117, :])
                    nc.sync.dma_start(
                        out=out4[qb * 117 : (qb + 1) * 117, :], in_=o_sb
                    )

            # Flat step loop with the MM2 group trailing one step GLOBALLY
            # (also across band boundaries): the PE FIFO never waits on a
            # band's last exp before starting the next band's MM1s.
            out_tiles = {}
            pending = None
            for s in range(NQ * (NPB // 4)):
                qb, pg = s // (NPB // 4), s % (NPB // 4)
                if pg == 0:
                    out_tiles[qb] = outps.tile([128, QB], f32, name="out_ps")
                a_t = []
                for half in range(2):  # 2 dot pairs of [128, 1024]
                    dot_ps = dotps.tile([128, 2 * QB], f32)
                    for j in range(2):
                        pb = pg * 4 + half * 2 + j
                        rt = half * 2 + j  # PE row-tile (K=32 packing x4)
                        if ROW_TILED_MM1:
                            nc.tensor.matmul(
                                dot_ps[:, j * QB : (j + 1) * QB],
                                lhsT=flt_s[
                                    32 * rt : 32 * rt + 32,
                                    pb * 128 : (pb + 1) * 128,
                                ],
                                rhs=frt_s[
                                    32 * rt : 32 * rt + 32,
                                    qb * QB : (qb + 1) * QB,
                                ],
                                tile_position=(32 * rt, 0),
                                start=True,
                                stop=True,
                            )
                        else:
                            nc.tensor.matmul(
                                dot_ps[:, j * QB : (j + 1) * QB],
                                lhsT=flt_s[:, pb * 128 : (pb + 1) * 128],
                                rhs=frt_s[:, qb * QB : (qb + 1) * QB],
                                start=True,
                                stop=True,
                            )
                    at = apool.tile([128, 2 * QB], bf16)
                    if _dve_tile(qb, pg, half):
                        # Schraudolph exp on DVE: the uint16 affine image
                        # of dot IS the bf16 bit pattern of ~exp(dot).
                        nc.vector.tensor_scalar(
                            at.bitcast(u16),
                            dot_ps,
                            SCH_C1,
                            SCH_C2,
                            mybir.AluOpType.mult,
                            mybir.AluOpType.add,
                        )
                    else:
                        nc.scalar.activation(
                            at, dot_ps, mybir.ActivationFunctionType.Exp
                        )
                    a_t.append(at)
                if pending is not None:
                    pqb, ppg, pa = pending
                    emit_mm2(out_tiles[pqb], ppg, pa)
                    if ppg == NPB // 4 - 1:
                        emit_band_out(pqb, out_tiles[pqb])
                pending = (qb, pg, a_t)
            pqb, ppg, pa = pending
            emit_mm2(out_tiles[pqb], ppg, pa)
            emit_band_out(pqb, out_tiles[pqb])

    _split_multi_waits(nc)
    _PROGRAM = nc
    return nc


def _host_prep(images, segmentations, ROIs, seg_label):
    """Resizes, gate, seg_m, bilateral features + hi/lo split. All fp32."""
    images = np.asarray(images, np.float32)
    segmentations = np.asarray(segmentations, np.float32)
    ROIs = np.asarray(ROIs, np.float32)
    seg_label = np.asarray(seg_label, np.float32)

    # nearest resize (scale 0.5, floor(dst*2)) == [::2, ::2]
    img_s = images[:, :, ::2, ::2]  # [N,3,64,64]
    roi_s = ROIs[:, ::2, ::2]  # [N,64,64]
    lab_s = seg_label[:, 0, ::2, ::2]  # [N,64,64]
    # bilinear (align_corners=False, scale 0.5) == 2x2 average pooling
    s = segmentations.reshape(N, K, OH, 2, OW, 2)
    seg_s = 0.25 * (s[:, :, :, 0, :, 0] + s[:, :, :, 0, :, 1]
                    + s[:, :, :, 1, :, 0] + s[:, :, :, 1, :, 1])

    unlabel = lab_s.astype(np.int32) == IGNORE_LABEL
    gate = roi_s - seg_s.max(axis=1)
    gate = np.where(unlabel, np.float32(1.0), gate)
    gate = np.maximum(gate, 0.0).reshape(N, P)  # [N,P]

    seg_m = (seg_s * roi_s[:, None]).reshape(N, K, P)  # [N,K,P]

    sxy = SIGMA_XY * SCALE
    ys, xs = np.meshgrid(np.arange(OH, dtype=np.float32),
                         np.arange(OW, dtype=np.float32), indexing="ij")
    xy = np.stack([xs.ravel(), ys.ravel()], axis=1) / sxy  # [P,2]
    rgb = img_s.reshape(N, 3, P).transpose(0, 2, 1) / SIGMA_RGB  # [N,P,3]
    feat = np.concatenate(
        [np.broadcast_to(xy, (N, P, 2)), rgb], axis=-1
    ).astype(np.float32)  # [N,P,5]

    sq = np.sum(feat * feat, axis=-1)  # [N,P]
    ones = np.ones((N, P, 1), np.float32)
    mhalf = (-0.5 * sq)[:, :, None]
    featL = np.concatenate([feat, ones, mhalf], axis=-1)  # [N,P,7]
    featR = np.concatenate([feat, mhalf, ones], axis=-1)  # [N,P,7]

    hiL, loL = _hilo(featL)
    hiR, loR = _hilo(featR)
    # 21 real contraction rows (+11 zero pad rows): dot = hiL.hiR + hiL.loR
    # + loL.hiR.
    fLT = np.zeros((N, CROWS, P), BF16)
    fRT = np.zeros((N, CROWS, P), BF16)
    fL21 = np.concatenate([hiL, hiL, loL], axis=-1).transpose(0, 2, 1)
    fR21 = np.concatenate([hiR, loR, hiR], axis=-1).transpose(0, 2, 1)
    reps = range(4) if ROW_TILED_MM1 else range(1)
    for rep in reps:  # replicas at partition 32*rep for PE row tiling
        fLT[:, 32 * rep : 32 * rep + 21] = fL21
        fRT[:, 32 * rep : 32 * rep + 21] = fR21

    # st arrangement [N, 128, NPB*21]
    st = (
        seg_m.astype(BF16)
        .transpose(0, 2, 1)  # [N,P,K]
        .reshape(N, NPB, 128, K)
        .transpose(0, 2, 1, 3)  # [N,128,NPB,K]
        .reshape(N, 128, NPB * K)
        .copy()
    )
    return seg_m, gate, fLT, fRT, st


def _in_maps(fLT, fRT, st):
    in_maps = []
    for c in range(N_CORES):
        n, half = c // 2, c % 2
        qs = slice(half * QCOLS, (half + 1) * QCOLS)
        in_maps.append(
            {
                "flt": np.ascontiguousarray(fLT[n]),
                "frt": np.ascontiguousarray(fRT[n][:, qs]),
                "st": st[n],
            }
        )
    return in_maps


def kernel(images, segmentations, ROIs, seg_label):
    from concourse.bass_utils import run_bass_kernel_spmd

    seg_m, gate, fLT, fRT, st = _host_prep(
        images, segmentations, ROIs, seg_label
    )

    nc = _build_program()
    res = run_bass_kernel_spmd(
        nc, _in_maps(fLT, fRT, st), core_ids=list(range(N_CORES))
    )

    AS = np.empty((N, K, P), np.float64)
    for c in range(N_CORES):
        n, half = c // 2, c % 2
        o4 = res.results[c]["out4"].astype(np.float64)  # [NQ*117, QB]
        for qb in range(NQ):
            o = o4[qb * 117 : (qb + 1) * 117]
            AS[n, :, half * QCOLS + qb * QB : half * QCOLS + (qb + 1) * QB] = (
                o[0:21] + o[32:53] + o[64:85] + o[96:117]
            )

    total = np.sum(
        seg_m.astype(np.float64) * gate[:, None].astype(np.float64) * AS
    )
    loss = WEIGHT * (-0.5) * total / (N * P)
    return np.array(loss, dtype=np.float32)


# revision 23
# speedup vs baseline: 1.0154x; 1.0154x over previous
"""Dense bilateral energy loss (DenseEnergyLoss) on 8 Trainium2 cores.

Math (per image n, after 2x downsample => oh=ow=64, P=4096):
  feat[p] = (x/40, y/40, r/15, g/15, b/15)          # 5 dims
  A[p,q]  = exp(-(||feat_p - feat_q||^2)/2)          # dense [P,P]
  AS[k,q] = sum_p seg_m[k,p] * A[p,q]                # A symmetric
  loss    = -0.05 * sum_{k,q} seg_m[k,q]*gate[q]*AS[k,q] / (N*P)

Device work per core (half an image: 2048 of the 4096 q columns):
  MM1 (PE):  dot[p,q] = -0.5*d2[p,q] via bf16 hi/lo-split contraction,
             zero-padded 21->128 rows (keeps PE HAM at 2.4GHz; K=32 without
             padding measured: HAM never unthrottles, PE stuck at 1.2GHz;
             4x row-tiled K=32 also measured: HAM stays cold and the row
             tiles mostly serialize -- net loss).
  EXP: split across two engines per [128,1024] dot tile:
    ACT: A = exp(dot) -> bf16  (exact spline)
    DVE: Schraudolph bit-trick: u16 = sat_round(dot*128*log2e + 128*(127-s));
         those 16 bits ARE the bf16 A up to a mean-zero sawtooth (<2e-3 on
         the final loss after calibrating s; verified in numpy + HW probe:
         the DVE fp32->uint16 converter rounds-to-nearest and saturates).
  MM2 (PE):  AS^T accumulation, col-tiled 4x (tile_position=(0,32j)).
  Per band: ACT copies the 4 col-group partials [117,512] PSUM->SBUF, DMA'd
  out; HOST sums the 4 partials (removes the DVE add-chain + fat tail).
Host (numpy): resizes, gate, seg_m, features, hi/lo split, partial sums,
final masked reduction.
"""

import sys

sys.path.insert(0, "/opt/trn_rl_repo")

import numpy as np
import ml_dtypes

# ---------------- problem constants (hardcoded per contract) ---------------
N, K, H, W = 4, 21, 128, 128
OH, OW = 64, 64
P = OH * OW  # 4096
WEIGHT = 0.1
SIGMA_RGB = 15.0
SIGMA_XY = 80.0
SCALE = 0.5
IGNORE_LABEL = 255
N_CORES = 8
QCOLS = P // 2  # q columns per core (2 cores per image)
QB = 512  # q tile width (one PSUM bank)
NQ = QCOLS // QB  # 4 q-bands per core
NPB = P // 128  # 32 p-blocks
CROWS = 128  # MM1 contraction rows (21 real + zero padding)
ROW_TILED_MM1 = False  # 4 concurrent K=32 row tiles (needs 4x replicated rows)

# Schraudolph constants (calibrated in numpy on the real inputs)
SCH_SIGMA = 0.0565
SCH_C1 = 128.0 / float(np.log(2.0))
SCH_C2 = 128.0 * (127.0 - SCH_SIGMA)

BF16 = ml_dtypes.bfloat16

_PROGRAM = None  # built once per process


def _dve_tile(qb, pg, half):
    """Which dot tiles the DVE (Schraudolph) handles vs ACT (exact exp).
    DVE gets half==1 except two tiles given back to ACT for balance."""
    if half != 0:
        return False
    if pg == 7 and qb in (1, 2):
        return False
    return True


def _hilo(x):
    """Split fp32 array into bf16 hi + bf16 lo with x ~= hi + lo."""
    x = np.asarray(x, np.float32)
    hi = x.astype(BF16)
    lo = (x - hi.astype(np.float32)).astype(BF16)
    return hi, lo


def _patch_tile_drain():
    """This container's walrus allows only one sync wait per CTRL (Drain/Nop)
    instruction; Tile's exit drain attaches one wait per DMA-HW queue sem.
    Split the extra waits onto dedicated nops."""
    from concourse import mybir
    from concourse.tile import TileContext
    from concourse.vector_clock import ScopedClock

    if getattr(TileContext, "_drain_split_patched", False):
        return

    def _drain_and_barrier(self, tick_clock, wait_clock):
        nc = self.nc
        drain_inst = nc.sync.drain()
        wait_clock.add_sem_waits(
            drain_inst.ins, ScopedClock({None: tick_clock.global_clock})
        )
        si = drain_inst.ins.sync_info
        waits = list(si.on_wait) if si is not None else []
        if len(waits) > 1:
            del si.on_wait[1:]
            for w in waits[1:]:
                n = nc.sync.nop(nofuse=True, hint="drain_split")
                n.ins.sync_info = mybir.SyncInfo(on_wait=[w], on_update=[])
        nc.all_engine_barrier()
        popped = nc._tile_sem_poison_stack.pop()
        assert popped is self._sem_poison
        nc.clear_and_free_semaphores(list(self.sems.allocated().values()))
        nc.all_engine_barrier()

    TileContext._drain_and_barrier = _drain_and_barrier
    TileContext._drain_split_patched = True


def _split_multi_waits(nc):
    """This walrus build supports one sync-wait per instruction. Hoist extra
    waits onto dedicated same-engine nops placed right before the owner."""
    from concourse import mybir

    ctr = 0
    for fn in nc.m.functions:
        for blk in fn.blocks:
            insts = blk.instructions
            new = []
            changed = False
            for inst in insts:
                si = inst.sync_info
                if si is not None and si.on_wait is not None and len(si.on_wait) > 1:
                    waits = list(si.on_wait)
                    for w in waits[:-1]:
                        ctr += 1
                        new.append(
                            mybir.InstNoOp(
                                name=f"WSPLIT-{ctr}",
                                engine=inst.engine,
                                ins=[],
                                outs=[],
                                sync_info=mybir.SyncInfo(
                                    on_wait=[w], on_update=[]
                                ),
                                text_hint="wait_split",
                                bass_nofuse=True,
                            )
                        )
                    si.on_wait = [waits[-1]]
                    inst.sync_info = si
                    changed = True
                new.append(inst)
            if changed:
                blk.instructions = new


def _build_program():
    global _PROGRAM
    if _PROGRAM is not None:
        return _PROGRAM

    _patch_tile_drain()
    import concourse.bass as bass
    from concourse import mybir
    from concourse.tile import TileContext

    nc = bass.Bass("TRN2")
    f32 = mybir.dt.float32
    bf16 = mybir.dt.bfloat16
    u16 = mybir.dt.uint16

    # Full 128 contraction rows (21 real + 107 host-zeroed): 128-partition
    # DMAs engage all 16 SDMA engines (32/96-row transfers measured ~4x
    # slower), and the zero pad rows keep the PE HAM activity monitor at
    # full clock.
    flt = nc.dram_tensor("flt", [CROWS, P], bf16, kind="ExternalInput")
    frt = nc.dram_tensor("frt", [CROWS, QCOLS], bf16, kind="ExternalInput")
    # seg_m^T pre-arranged [128, NPB*21]: st[p, pb*21+k] = seg_m[k, pb*128+p]
    st = nc.dram_tensor("st", [128, NPB * 21], bf16, kind="ExternalInput")
    # per-band 4 col-group partials [117, QB], host sums rows 0:21,32:53,...
    out4 = nc.dram_tensor("out4", [NQ * 117, QB], f32, kind="ExternalOutput")

    with TileContext(nc) as tc:
        with (
            tc.tile_pool(name="const", bufs=1) as const,
            tc.tile_pool(name="apool", bufs=10) as apool,
            tc.tile_pool(name="osb", bufs=2) as osb,
            tc.tile_pool(name="dotps", bufs=3, space="PSUM") as dotps,
            tc.tile_pool(name="outps", bufs=2, space="PSUM") as outps,
        ):
            flt_s = const.tile([CROWS, P], bf16)
            frt_s = const.tile([CROWS, QCOLS], bf16)
            st_s = const.tile([128, NPB * 21], bf16)
            # Inputs on two HWDGE rings in parallel (sync + scalar),
            # column-chunked so the first band/p-blocks land earliest.
            nc.sync.dma_start(out=frt_s[:, 0:QB], in_=frt[:, 0:QB])
            nc.sync.dma_start(out=st_s, in_=st[:, :])
            nc.sync.dma_start(out=frt_s[:, QB:QCOLS], in_=frt[:, QB:QCOLS])
            for ck in range(4):  # 4 even chunks stay ahead of consumption
                nc.scalar.dma_start(
                    out=flt_s[:, ck * 1024 : (ck + 1) * 1024],
                    in_=flt[:, ck * 1024 : (ck + 1) * 1024],
                )

            def emit_mm2(out_ps, pg, a_pair):
                for j in range(4):  # col-tiled MM2, 4 p-blocks at once
                    pb = pg * 4 + j
                    nc.tensor.matmul(
                        out_ps[32 * j : 32 * j + 21, :],
                        lhsT=st_s[:, pb * 21 : (pb + 1) * 21],
                        rhs=a_pair[j // 2][:, (j % 2) * QB : (j % 2 + 1) * QB],
                        tile_position=(0, 32 * j),
                        start=(pg == 0),
                        stop=(pg == NPB // 4 - 1),
                    )

            def emit_band_out(qb, out_ps):
                # Evacuate the 4 col-group partials via ACT (close to PSUM);
                # host sums them. DMA on the sync ring (idle mid-loop).
                # Last band: split by ROWS across ACT and DVE in parallel
                # (contiguous DMA destinations; a column split was measured
                # 2.5us DMA-completion due to strided rows, gating the drain).
                if qb == NQ - 1:
                    o_sb1 = osb.tile([96, QB], f32, name="o_sb1", tag="of1")
                    nc.scalar.copy(o_sb1, out_ps[0:96, :])
                    nc.sync.dma_start(
                        out=out4[qb * 117 : qb * 117 + 96, :], in_=o_sb1
                    )
                    o_sb2 = osb.tile([21, QB], f32, name="o_sb2", tag="of2")
                    nc.vector.tensor_copy(o_sb2, out_ps[96:117, :])
                    nc.sync.dma_start(
                        out=out4[qb * 117 + 96 : (qb + 1) * 117, :], in_=o_sb2
                    )
                else:
                    o_sb = osb.tile(
                        [117, QB], f32, name="o_sb", tag=f"o{qb % 2}"
                    )
                    # qb 1,2 evacuate via DVE to balance engine load (ACT
                    # carries 34 exps vs DVE 30).
                    if qb in (1, 2):
                        nc.vector.tensor_copy(o_sb, out_ps[0:117, :])
                    else:
                        nc.scalar.copy(o_sb, out_ps[0:117, :])
                    nc.sync.dma_start(
                        out=out4[qb * 117 : (qb + 1) * 117, :], in_=o_sb
                    )

            # Flat step loop with the MM2 group trailing one step GLOBALLY
            # (also across band boundaries): the PE FIFO never waits on a
            # band's last exp before starting the next band's MM1s.
            out_tiles = {}
            pending = None
            for s in range(NQ * (NPB // 4)):
                qb, pg = s // (NPB // 4), s % (NPB // 4)
                if pg == 0:
                    out_tiles[qb] = outps.tile([128, QB], f32, name="out_ps")
                a_t = []
                for half in range(2):  # 2 dot pairs of [128, 1024]
                    dot_ps = dotps.tile([128, 2 * QB], f32)
                    for j in range(2):
                        pb = pg * 4 + half * 2 + j
                        rt = half * 2 + j  # PE row-tile (K=32 packing x4)
                        if ROW_TILED_MM1:
                            nc.tensor.matmul(
                                dot_ps[:, j * QB : (j + 1) * QB],
                                lhsT=flt_s[
                                    32 * rt : 32 * rt + 32,
                                    pb * 128 : (pb + 1) * 128,
                                ],
                                rhs=frt_s[
                                    32 * rt : 32 * rt + 32,
                                    qb * QB : (qb + 1) * QB,
                                ],
                                tile_position=(32 * rt, 0),
                                start=True,
                                stop=True,
                            )
                        else:
                            nc.tensor.matmul(
                                dot_ps[:, j * QB : (j + 1) * QB],
                                lhsT=flt_s[:, pb * 128 : (pb + 1) * 128],
                                rhs=frt_s[:, qb * QB : (qb + 1) * QB],
                                start=True,
                                stop=True,
                            )
                    at = apool.tile([128, 2 * QB], bf16)
                    if _dve_tile(qb, pg, half):
                        # Schraudolph exp on DVE: the uint16 affine image
                        # of dot IS the bf16 bit pattern of ~exp(dot).
                        nc.vector.tensor_scalar(
                            at.bitcast(u16),
                            dot_ps,
                            SCH_C1,
                            SCH_C2,
                            mybir.AluOpType.mult,
                            mybir.AluOpType.add,
                        )
                    else:
                        nc.scalar.activation(
                            at, dot_ps, mybir.ActivationFunctionType.Exp
                        )
                    a_t.append(at)
                if pending is not None:
                    pqb, ppg, pa = pending
                    emit_mm2(out_tiles[pqb], ppg, pa)
                    if ppg == NPB // 4 - 1:
                        emit_band_out(pqb, out_tiles[pqb])
                pending = (qb, pg, a_t)
            pqb, ppg, pa = pending
            emit_mm2(out_tiles[pqb], ppg, pa)
            emit_band_out(pqb, out_tiles[pqb])

    _split_multi_waits(nc)
    _PROGRAM = nc
    return nc


def _host_prep(images, segmentations, ROIs, seg_label):
    """Resizes, gate, seg_m, bilateral features + hi/lo split. All fp32."""
    images = np.asarray(images, np.float32)
    segmentations = np.asarray(segmentations, np.float32)
    ROIs = np.asarray(ROIs, np.float32)
    seg_label = np.asarray(seg_label, np.float32)

    # nearest resize (scale 0.5, floor(dst*2)) == [::2, ::2]
    img_s = images[:, :, ::2, ::2]  # [N,3,64,64]
    roi_s = ROIs[:, ::2, ::2]  # [N,64,64]
    lab_s = seg_label[:, 0, ::2, ::2]  # [N,64,64]
    # bilinear (align_corners=False, scale 0.5) == 2x2 average pooling
    s = segmentations.reshape(N, K, OH, 2, OW, 2)
    seg_s = 0.25 * (s[:, :, :, 0, :, 0] + s[:, :, :, 0, :, 1]
                    + s[:, :, :, 1, :, 0] + s[:, :, :, 1, :, 1])

    unlabel = lab_s.astype(np.int32) == IGNORE_LABEL
    gate = roi_s - seg_s.max(axis=1)
    gate = np.where(unlabel, np.float32(1.0), gate)
    gate = np.maximum(gate, 0.0).reshape(N, P)  # [N,P]

    seg_m = (seg_s * roi_s[:, None]).reshape(N, K, P)  # [N,K,P]

    sxy = SIGMA_XY * SCALE
    ys, xs = np.meshgrid(np.arange(OH, dtype=np.float32),
                         np.arange(OW, dtype=np.float32), indexing="ij")
    xy = np.stack([xs.ravel(), ys.ravel()], axis=1) / sxy  # [P,2]
    rgb = img_s.reshape(N, 3, P).transpose(0, 2, 1) / SIGMA_RGB  # [N,P,3]
    feat = np.concatenate(
        [np.broadcast_to(xy, (N, P, 2)), rgb], axis=-1
    ).astype(np.float32)  # [N,P,5]

    sq = np.sum(feat * feat, axis=-1)  # [N,P]
    ones = np.ones((N, P, 1), np.float32)
    mhalf = (-0.5 * sq)[:, :, None]
    featL = np.concatenate([feat, ones, mhalf], axis=-1)  # [N,P,7]
    featR = np.concatenate([feat, mhalf, ones], axis=-1)  # [N,P,7]

    hiL, loL = _hilo(featL)
    hiR, loR = _hilo(featR)
    # 21 real contraction rows (+11 zero pad rows): dot = hiL.hiR + hiL.loR
    # + loL.hiR.
    fLT = np.zeros((N, CROWS, P), BF16)
    fRT = np.zeros((N, CROWS, P), BF16)
    fL21 = np.concatenate([hiL, hiL, loL], axis=-1).transpose(0, 2, 1)
    fR21 = np.concatenate([hiR, loR, hiR], axis=-1).transpose(0, 2, 1)
    reps = range(4) if ROW_TILED_MM1 else range(1)
    for rep in reps:  # replicas at partition 32*rep for PE row tiling
        fLT[:, 32 * rep : 32 * rep + 21] = fL21
        fRT[:, 32 * rep : 32 * rep + 21] = fR21

    # st arrangement [N, 128, NPB*21]
    st = (
        seg_m.astype(BF16)
        .transpose(0, 2, 1)  # [N,P,K]
        .reshape(N, NPB, 128, K)
        .transpose(0, 2, 1, 3)  # [N,128,NPB,K]
        .reshape(N, 128, NPB * K)
        .copy()
    )
    return seg_m, gate, fLT, fRT, st


def _in_maps(fLT, fRT, st):
    in_maps = []
    for c in range(N_CORES):
        n, half = c // 2, c % 2
        qs = slice(half * QCOLS, (half + 1) * QCOLS)
        in_maps.append(
            {
                "flt": np.ascontiguousarray(fLT[n]),
                "frt": np.ascontiguousarray(fRT[n][:, qs]),
                "st": st[n],
            }
        )
    return in_maps


def kernel(images, segmentations, ROIs, seg_label):
    from concourse.bass_utils import run_bass_kernel_spmd

    seg_m, gate, fLT, fRT, st = _host_prep(
        images, segmentations, ROIs, seg_label
    )

    nc = _build_program()
    res = run_bass_kernel_spmd(
        nc, _in_maps(fLT, fRT, st), core_ids=list(range(N_CORES))
    )

    AS = np.empty((N, K, P), np.float64)
    for c in range(N_CORES):
        n, half = c // 2, c % 2
        o4 = res.results[c]["out4"].astype(np.float64)  # [NQ*117, QB]
        for qb in range(NQ):
            o = o4[qb * 117 : (qb + 1) * 117]
            AS[n, :, half * QCOLS + qb * QB : half * QCOLS + (qb + 1) * QB] = (
                o[0:21] + o[32:53] + o[64:85] + o[96:117]
            )

    total = np.sum(
        seg_m.astype(np.float64) * gate[:, None].astype(np.float64) * AS
    )
    loss = WEIGHT * (-0.5) * total / (N * P)
    return np.array(loss, dtype=np.float32)
